# revision 1
# baseline (speedup 1.0000x reference)
import sys

sys.path.insert(0, "/opt/trn_rl_repo")
import numpy as np
import concourse.bass as bass
import concourse.bacc as bacc
import concourse.mybir as mybir
import concourse.tile as tile
from concourse import masks
import concourse.bass_utils as bass_utils

bass_utils.upload_artifacts = lambda tmpdir: "local://" + tmpdir
from concourse.bass_utils import run_bass_kernel_spmd

N_CORES = 8
B, H, W, C, R = 32, 56, 56, 256, 16
BS = B // N_CORES          # 4 samples per core
NP = H * W                 # 3136 pixels per sample
NT = 25                    # tiles per sample: 24 x 128 + 1 x 64
ROWS = BS * NP             # 12544 rows per core
F32 = mybir.dt.float32
AL = mybir.AluOpType
AF = mybir.ActivationFunctionType
AX = mybir.AxisListType

_COMPILED = None


def _build():
    nc = bacc.Bacc(None, target_bir_lowering=False, num_devices=N_CORES)
    x_d = nc.declare_dram_parameter("x", [ROWS, C], F32, isOutput=False)
    w1_d = nc.declare_dram_parameter("w1", [C, R], F32, isOutput=False)
    b1_d = nc.declare_dram_parameter("b1", [1, R], F32, isOutput=False)
    w2_d = nc.declare_dram_parameter("w2", [R, C], F32, isOutput=False)
    b2_d = nc.declare_dram_parameter("b2", [1, C], F32, isOutput=False)
    wf_d = nc.declare_dram_parameter("wflat", [98, 1], F32, isOutput=False)
    bc_d = nc.declare_dram_parameter("bconv", [1, 1], F32, isOutput=False)
    out_d = nc.declare_dram_parameter("out", [ROWS, C], F32, isOutput=True)

    flat_dram = nc.dram_tensor("flat_dram", [2 * BS, NP], F32)
    fpad_dram = nc.dram_tensor("fpad_dram", [2 * BS, 3844], F32)

    with tile.TileContext(nc) as tc:
        with tc.tile_pool(name="const", bufs=1) as cp, \
             tc.tile_pool(name="xbuf", bufs=1) as xp, \
             tc.tile_pool(name="work", bufs=3) as wp, \
             tc.tile_pool(name="sp", bufs=3) as spp, \
             tc.tile_pool(name="psA", bufs=2, space="PSUM") as psA, \
             tc.tile_pool(name="psB", bufs=3, space="PSUM") as psB, \
             tc.tile_pool(name="psC", bufs=3, space="PSUM") as psC:

            # ---------- constants ----------
            ident = cp.tile([128, 128], F32)
            masks.make_identity(nc, ident[:])
            ones2 = cp.tile([2, 128], F32)
            nc.gpsimd.memset(ones2[:], 1.0)

            w1t = cp.tile([128, 2 * R], F32)       # [K-chunk, 2*16]
            nc.sync.dma_start(w1t[:, 0:R], w1_d[0:128, :])
            nc.sync.dma_start(w1t[:, R:2 * R], w1_d[128:256, :])
            w2t = cp.tile([R, C], F32)
            nc.sync.dma_start(w2t[:], w2_d[:])
            wf_t = cp.tile([98, 1], F32)
            nc.sync.dma_start(wf_t[:], wf_d[:])

            b1r = cp.tile([1, R], F32)
            nc.sync.dma_start(b1r[:], b1_d[:])
            b1b = cp.tile([2, R], F32)
            nc.gpsimd.partition_broadcast(b1b[:], b1r[:], channels=2)
            b2r = cp.tile([1, C], F32)
            nc.sync.dma_start(b2r[:], b2_d[:])
            b2b = cp.tile([2, C], F32)
            nc.gpsimd.partition_broadcast(b2b[:], b2r[:], channels=2)
            bcr = cp.tile([1, 1], F32)
            nc.sync.dma_start(bcr[:], bc_d[:])
            bcb = cp.tile([128, 1], F32)
            nc.gpsimd.partition_broadcast(bcb[:], bcr[:], channels=128)

            # zero the padded-plane dram scratch (borders stay zero forever)
            zrow = cp.tile([2 * BS, 3844], F32)
            nc.vector.memset(zrow[:], 0.0)
            nc.sync.dma_start(fpad_dram.ap(), zrow[:])

            # resident x (overwritten in place by xg then by out)
            xbuf = xp.tile([128, BS * NT * C], F32)

            def xt(s, t):
                pt = 64 if t == NT - 1 else 128
                return xbuf[0:pt, (s * NT + t) * C:(s * NT + t + 1) * C]

            def xrows(s, t):
                r0 = s * NP + t * 128
                pt = 64 if t == NT - 1 else 128
                return x_d[r0:r0 + pt, :], out_d[r0:r0 + pt, :]

            maxacc_l, rhs_l, cb_l, spm_l, spx_l, spsc_l = {}, {}, {}, {}, {}, {}

            for s in range(BS):
                for t in range(NT):
                    src, _ = xrows(s, t)
                    nc.sync.dma_start(xt(s, t), src)

            for s in range(BS):
                # ============ phase A: load + pooling stats ============
                acc0 = psA.tile([128, 128], F32, tag="acc")
                acc1 = psA.tile([128, 128], F32, tag="acc")
                accs = [acc0, acc1]
                maxacc = wp.tile([128, C], F32, tag="maxacc")
                for t in range(NT):
                    pt = 64 if t == NT - 1 else 128
                    v = xt(s, t)
                    for c in range(2):
                        nc.tensor.matmul(
                            accs[c][:, 0:pt], v[:, c * 128:(c + 1) * 128],
                            ident[0:pt, 0:pt], is_transpose=True,
                            start=(t == 0), stop=(t == NT - 1),
                            skip_group_check=True)
                    if t == 0:
                        nc.vector.tensor_copy(maxacc[:], v)
                    else:
                        nc.vector.tensor_tensor(
                            out=maxacc[0:pt, :], in0=maxacc[0:pt, :], in1=v,
                            op=AL.max)

                # ============ phase A finalize: pooled vectors ============
                rhs_s = wp.tile([128, 4], F32, tag="rhs")
                for c in range(2):
                    tmp = wp.tile([128, 1], F32, tag="redtmp")
                    nc.vector.reduce_sum(tmp[:], accs[c][:], axis=AX.X)
                    nc.scalar.activation(rhs_s[:, 2 * c:2 * c + 1], tmp[:],
                                         AF.Copy, scale=1.0 / NP)
                    mt = psB.tile([128, 128], F32, tag="psb")
                    nc.tensor.transpose(mt[:], maxacc[:, c * 128:(c + 1) * 128],
                                        ident[:])
                    nc.vector.reduce_max(rhs_s[:, 2 * c + 1:2 * c + 2], mt[:],
                                         axis=AX.X)

                # ============ phase B: MLP -> channel scale row ============
                h_ps = psB.tile([2, R], F32, tag="psb")
                nc.tensor.matmul(h_ps[:], rhs_s[:, 0:2], w1t[:, 0:R],
                                 start=True, stop=False)
                nc.tensor.matmul(h_ps[:], rhs_s[:, 2:4], w1t[:, R:2 * R],
                                 start=False, stop=True)
                hb = wp.tile([2, R], F32, tag="hb")
                nc.vector.tensor_tensor(out=hb[:], in0=h_ps[:], in1=b1b[:],
                                        op=AL.add)
                hr = wp.tile([2, R], F32, tag="hr")
                nc.scalar.activation(hr[:], hb[:], AF.Relu)
                hT_ps = psB.tile([R, 2], F32, tag="psb")
                nc.tensor.transpose(hT_ps[:], hr[:], ident[0:2, 0:2])
                hT = wp.tile([R, 2], F32, tag="hT")
                nc.vector.tensor_copy(hT[:], hT_ps[:])
                co_ps = psB.tile([2, C], F32, tag="psb")
                nc.tensor.matmul(co_ps[:], hT[:], w2t[:], start=True, stop=True)
                co_sb = wp.tile([2, C], F32, tag="co")
                nc.vector.tensor_tensor(out=co_sb[:], in0=co_ps[:], in1=b2b[:],
                                        op=AL.add)
                sig = wp.tile([2, C], F32, tag="sig")
                nc.scalar.activation(sig[:], co_sb[:], AF.Sigmoid)
                cb_ps = psB.tile([128, C], F32, tag="psb")
                nc.tensor.matmul(cb_ps[:], ones2[:], sig[:], start=True, stop=True)
                cb = wp.tile([128, C], F32, tag="cb")
                nc.vector.tensor_copy(cb[:], cb_ps[:])

                # ============ phase C: xg (in place) + spatial stats ============
                spm = spp.tile([128, NT], F32, tag="spm")
                spx = spp.tile([128, NT], F32, tag="spx")
                nc.vector.memset(spm[64:128, NT - 1:NT], 0.0)
                nc.vector.memset(spx[64:128, NT - 1:NT], 0.0)
                for t in range(NT):
                    pt = 64 if t == NT - 1 else 128
                    v = xt(s, t)
                    nc.vector.tensor_tensor(out=v, in0=v, in1=cb[0:pt, :],
                                            op=AL.mult)
                    nc.vector.reduce_max(spx[0:pt, t:t + 1], v, axis=AX.X)
                    nc.scalar.activation(v, v, AF.Copy,
                                         accum_out=spm[0:pt, t:t + 1])

                # ============ phase D: 7x7x2 conv via patch matmuls ============
                for c, sp_t in enumerate((spm, spx)):
                    row = s * 2 + c
                    spT_ps = psB.tile([NT, 128], F32, tag="psb")
                    nc.tensor.transpose(spT_ps[:], sp_t[:], ident[:])
                    spT = wp.tile([NT, 128], F32, tag="spT")
                    nc.vector.tensor_copy(spT[:], spT_ps[:])
                    nc.sync.dma_start(
                        bass.AP(flat_dram, row * NP, [[128, 24], [1, 128]]),
                        spT[0:24, :])
                    nc.sync.dma_start(
                        bass.AP(flat_dram, row * NP + 3072, [[1, 64]]),
                        spT[24:25, 0:64])
                    nc.sync.dma_start(
                        bass.AP(fpad_dram, row * 3844 + 3 * 62 + 3,
                                [[62, 56], [1, 56]]),
                        bass.AP(flat_dram, row * NP, [[56, 56], [1, 56]]))
                patches = wp.tile([98, NP], F32, tag="patches")
                for c in range(2):
                    row = s * 2 + c
                    for dy in range(7):
                        nc.sync.dma_start(
                            patches[c * 49 + dy * 7:c * 49 + dy * 7 + 7, :],
                            bass.AP(fpad_dram, row * 3844 + dy * 62,
                                    [[1, 7], [62, 56], [1, 56]]))
                conv_ps = psC.tile([128, NT], F32, tag="conv")
                for t in range(NT):
                    pt = 64 if t == NT - 1 else 128
                    nc.tensor.matmul(conv_ps[0:pt, t:t + 1],
                                     patches[:, t * 128:t * 128 + pt],
                                     wf_t[:], start=True, stop=True,
                                     skip_group_check=True)
                nc.vector.memset(conv_ps[64:128, NT - 1:NT], 0.0)
                spsc = spp.tile([128, NT], F32, tag="spsc")
                nc.scalar.activation(spsc[:], conv_ps[:], AF.Sigmoid, bias=bcb[:])

                # ============ phase E: out = xg * spatial, store ============
                for t in range(NT):
                    pt = 64 if t == NT - 1 else 128
                    v = xt(s, t)
                    _, dst = xrows(s, t)
                    nc.scalar.activation(v, v, AF.Copy,
                                         scale=spsc[0:pt, t:t + 1])
                    nc.sync.dma_start(dst, v)

    nc.compile()
    return nc


def _get_compiled():
    global _COMPILED
    if _COMPILED is None:
        _COMPILED = _build()
    return _COMPILED


def kernel(x, w1, b1, w2, b2, wconv, bconv):
    x = np.ascontiguousarray(np.asarray(x, dtype=np.float32))
    # wconv [7,7,2,1] -> wflat[k] = wconv[dy,dx,c]; k = c*49 + dy*7 + dx
    wf = np.asarray(wconv, dtype=np.float32)[:, :, :, 0].transpose(2, 0, 1).copy()
    wf[0] /= C          # fold the channel-mean (1/256) into the conv weight
    wf = wf.reshape(98, 1)

    nc = _get_compiled()
    xs = x.reshape(N_CORES, ROWS, C)
    in_maps = [{
        "x": xs[i],
        "w1": np.asarray(w1, np.float32),
        "b1": np.asarray(b1, np.float32).reshape(1, R),
        "w2": np.asarray(w2, np.float32),
        "b2": np.asarray(b2, np.float32).reshape(1, C),
        "wflat": wf,
        "bconv": np.asarray(bconv, np.float32).reshape(1, 1),
    } for i in range(N_CORES)]
    res = run_bass_kernel_spmd(nc, in_maps, list(range(N_CORES)))
    out = np.stack([res.results[i]["out"] for i in range(N_CORES)], axis=0)
    return out.reshape(B, H, W, C)



# revision 12
# speedup vs baseline: 1.4941x; 1.4941x over previous
import sys

sys.path.insert(0, "/opt/trn_rl_repo")
import numpy as np
import ml_dtypes
import concourse.bass as bass
import concourse.bacc as bacc
import concourse.mybir as mybir
import concourse.tile as tile
from concourse import masks
import concourse.bass_utils as bass_utils

bass_utils.upload_artifacts = lambda tmpdir: "local://" + tmpdir
from concourse.bass_utils import run_bass_kernel_spmd

N_CORES = 8
B, H, W, C, R = 32, 56, 56, 256, 16
BS = B // N_CORES          # 4 samples per core
NP = H * W                 # 3136 pixels per sample
PT = 112                   # partitions per tile (2 image rows)
NT = NP // PT              # 28 tiles per sample
TPS = 7                    # tiles per DMA slab
NSLAB = NT // TPS          # 4 slabs per sample
ROWS = BS * NP             # 12544 rows per core
F32 = mybir.dt.float32
BF16 = mybir.dt.bfloat16
AL = mybir.AluOpType
AF = mybir.ActivationFunctionType
AX = mybir.AxisListType

_COMPILED = None


def _build():
    nc = bacc.Bacc(None, target_bir_lowering=False, num_devices=N_CORES)
    x_d = nc.declare_dram_parameter("x", [ROWS, C], BF16, isOutput=False)
    w1_d = nc.declare_dram_parameter("w1", [C, R], F32, isOutput=False)
    b1_d = nc.declare_dram_parameter("b1", [1, R], F32, isOutput=False)
    w2_d = nc.declare_dram_parameter("w2", [R, C], F32, isOutput=False)
    b2_d = nc.declare_dram_parameter("b2", [1, C], F32, isOutput=False)
    wv_d = nc.declare_dram_parameter("wv", [14 * 62, 56], F32, isOutput=False)
    bc_d = nc.declare_dram_parameter("bconv", [1, 1], F32, isOutput=False)
    out_d = nc.declare_dram_parameter("out", [ROWS, C], BF16, isOutput=True)
    # DRAM bounce buffers for plane-layout rearrangement (ping-pong x2)
    sd_dram = nc.dram_tensor("sd_dram", [2 * 2, NP], F32)
    sc_dram = nc.dram_tensor("sc_dram", [2, NP], F32)

    with tile.TileContext(nc) as tc:
        with tc.tile_pool(name="const", bufs=1) as cp, \
             tc.tile_pool(name="xbuf", bufs=1) as xp, \
             tc.tile_pool(name="maxp", bufs=4) as mp, \
             tc.tile_pool(name="work", bufs=2) as wp, \
             tc.tile_pool(name="sp", bufs=2) as spp, \
             tc.tile_pool(name="psPool", bufs=2, space="PSUM") as psP, \
             tc.tile_pool(name="psA", bufs=2, space="PSUM") as psA, \
             tc.tile_pool(name="psB", bufs=2, space="PSUM") as psB, \
             tc.tile_pool(name="psC", bufs=2, space="PSUM") as psC:

            # ---------- constants ----------
            ident_f = cp.tile([128, 128], F32)
            masks.make_identity(nc, ident_f[:])
            ident_b = cp.tile([128, 128], BF16)
            masks.make_identity(nc, ident_b[:])
            ones_t = cp.tile([PT, 1], BF16)
            nc.gpsimd.memset(ones_t[:], 1.0)
            ones2f = cp.tile([2, PT], F32)
            nc.gpsimd.memset(ones2f[:], 1.0)

            w1t = cp.tile([128, 2 * R], F32)       # [K-chunk, 2*16]
            nc.sync.dma_start(w1t[:, 0:R], w1_d[0:128, :])
            nc.sync.dma_start(w1t[:, R:2 * R], w1_d[128:256, :])
            w2t = cp.tile([R, C], F32)
            nc.sync.dma_start(w2t[:], w2_d[:])
            # 14 band matrices [62,56], one per (channel, dx)
            wv_sb = cp.tile([62, 14, 56], F32)
            nc.sync.dma_start(
                wv_sb[:],
                bass.AP(wv_d, 0, [[56, 62], [62 * 56, 14], [1, 56]]))

            b1r = cp.tile([1, R], F32)
            nc.sync.dma_start(b1r[:], b1_d[:])
            b1b = cp.tile([2, R], F32)
            nc.gpsimd.partition_broadcast(b1b[:], b1r[:], channels=2)
            b2r = cp.tile([1, C], F32)
            nc.sync.dma_start(b2r[:], b2_d[:])
            b2b = cp.tile([2, C], F32)
            nc.gpsimd.partition_broadcast(b2b[:], b2r[:], channels=2)
            bcr = cp.tile([1, 1], F32)
            nc.sync.dma_start(bcr[:], bc_d[:])
            bcb = cp.tile([56, 1], F32)
            nc.gpsimd.partition_broadcast(bcb[:], bcr[:], channels=56)

            # zero-padded conv input planes (borders stay zero), ping-pong x2
            pads = []
            for i in range(2):
                pm = cp.tile([62, 56], F32, name=f"padm{i}")
                px = cp.tile([62, 56], F32, name=f"padx{i}")
                nc.vector.memset(pm[:], 0.0)
                nc.vector.memset(px[:], 0.0)
                pads.append((pm, px))

            # resident x (overwritten in place by xg then by out)
            xbuf = xp.tile([PT, BS * NT, C], BF16)

            # ---------- load all of x ----------
            for s in range(BS):
                for j in range(NSLAB):
                    base = (s * NP + j * TPS * PT) * C
                    nc.sync.dma_start(
                        xbuf[:, s * NT + j * TPS:s * NT + (j + 1) * TPS, :],
                        bass.AP(x_d, base, [[C, PT], [PT * C, TPS], [1, C]]))

            for s in range(BS):
                # ---------- channel pooling ----------
                pool_ps = psP.tile([1, C], F32, tag="pool")
                for t in range(NT):
                    v = xbuf[:, s * NT + t, :]
                    nc.tensor.matmul(
                        pool_ps[:], ones_t[:], v,
                        start=(t == 0), stop=(t == NT - 1),
                        skip_group_check=True)
                scr = wp.tile([PT, 14, C], BF16, tag="scr")
                nc.vector.tensor_tensor(
                    out=scr[:], in0=xbuf[:, s * NT:s * NT + 14, :],
                    in1=xbuf[:, s * NT + 14:s * NT + 28, :], op=AL.max)
                s7 = wp.tile([PT, 7, C], BF16, tag="s7")
                nc.vector.tensor_tensor(
                    out=s7[:], in0=scr[:, 0:7, :], in1=scr[:, 7:14, :],
                    op=AL.max)
                maxacc = wp.tile([PT, C], BF16, tag="maxacc")
                nc.vector.tensor_tensor(out=maxacc[:], in0=s7[:, 0, :],
                                        in1=s7[:, 1, :], op=AL.max)
                for k in range(2, 7):
                    nc.vector.tensor_tensor(out=maxacc[:], in0=maxacc[:],
                                            in1=s7[:, k, :], op=AL.max)

                # ---------- channel MLP ----------
                poolsb = wp.tile([1, C], F32, tag="poolsb")
                nc.scalar.activation(poolsb[:], pool_ps[:],
                                     AF.Copy, scale=1.0 / NP)
                rhs_s = wp.tile([128, 2, 2], F32, tag="rhs")
                for c in range(2):
                    tp = psB.tile([128, 1], F32, tag="psb")
                    nc.tensor.transpose(tp[:], poolsb[:, c * 128:(c + 1) * 128],
                                        ident_f[0:1, 0:1])
                    nc.vector.tensor_copy(rhs_s[:, c, 0:1], tp[:])
                    mt = psA.tile([128, PT], BF16, tag="psa")
                    nc.tensor.transpose(mt[:], maxacc[:, c * 128:(c + 1) * 128],
                                        ident_b[0:PT, 0:PT])
                    nc.vector.reduce_max(rhs_s[:, c, 1:2], mt[:], axis=AX.X)
                h_ps = psB.tile([2, R], F32, tag="psb")
                nc.tensor.matmul(h_ps[:], rhs_s[:, 0, :], w1t[:, 0:R],
                                 start=True, stop=False)
                nc.tensor.matmul(h_ps[:], rhs_s[:, 1, :], w1t[:, R:2 * R],
                                 start=False, stop=True)
                hb = wp.tile([2, R], F32, tag="hb")
                nc.vector.tensor_tensor(out=hb[:], in0=h_ps[:], in1=b1b[:],
                                        op=AL.add)
                hr = wp.tile([2, R], F32, tag="hr")
                nc.scalar.activation(hr[:], hb[:], AF.Relu)
                hT_ps = psB.tile([R, 2], F32, tag="psb")
                nc.tensor.transpose(hT_ps[:], hr[:], ident_f[0:2, 0:2])
                hT = wp.tile([R, 2], F32, tag="hT")
                nc.vector.tensor_copy(hT[:], hT_ps[:])
                co_ps = psB.tile([2, C], F32, tag="psb")
                nc.tensor.matmul(co_ps[:], hT[:], w2t[:], start=True, stop=True)
                co_sb = wp.tile([2, C], F32, tag="co")
                nc.vector.tensor_tensor(out=co_sb[:], in0=co_ps[:], in1=b2b[:],
                                        op=AL.add)
                sig = wp.tile([2, C], F32, tag="sig")
                nc.scalar.activation(sig[:], co_sb[:], AF.Sigmoid)
                cb_ps = psB.tile([PT, C], F32, tag="psb")
                nc.tensor.matmul(cb_ps[:], ones2f[:], sig[:],
                                 start=True, stop=True)
                cbb = wp.tile([PT, C], BF16, tag="cbb")
                nc.vector.tensor_copy(cbb[:], cb_ps[:])

                # ---------- xg (in place) + spatial stats ----------
                spx = spp.tile([PT, NT], F32, tag="spx")
                spm = spp.tile([PT, NT], F32, tag="spm")
                for t in range(NT):
                    v = xbuf[:, s * NT + t, :]
                    nc.vector.tensor_tensor(out=v, in0=v, in1=cbb[:],
                                            op=AL.mult)
                    nc.vector.reduce_max(spx[:, t:t + 1], v, axis=AX.X)
                for j in range(NSLAB):
                    nc.vector.reduce_sum(
                        spm[:, j * TPS:(j + 1) * TPS],
                        xbuf[:, s * NT + j * TPS:s * NT + (j + 1) * TPS, :],
                        axis=AX.X)

                # ---------- 7x7x2 conv via banded matmuls ----------
                padm, padx = pads[s % 2]
                for ci, (plane, padt) in enumerate(((spm, padm), (spx, padx))):
                    tps = psA.tile([NT, PT], F32, tag="psa")
                    nc.tensor.transpose(tps[:], plane[:], ident_f[0:PT, 0:PT])
                    smT = wp.tile([NT, PT], F32, tag="smT")
                    nc.vector.tensor_copy(smT[:], tps[:])
                    row = (s % 2) * 2 + ci
                    nc.sync.dma_start(
                        bass.AP(sd_dram, row * NP, [[112, 28], [1, 112]]),
                        smT[:])
                    nc.sync.dma_start(
                        padt[3:59, :],
                        bass.AP(sd_dram, row * NP, [[56, 56], [1, 56]]))
                conv_ps = psC.tile([56, 56], F32, tag="conv")
                dx_order = [3, 0, 1, 2, 4, 5, 6]
                nmm = 0
                for c, padt in ((0, padm), (1, padx)):
                    for dx in (dx_order if c == 0 else range(7)):
                        d = dx - 3
                        a = max(0, -d)
                        b = 56 - max(0, d)
                        nc.tensor.matmul(
                            conv_ps[0:56, a:b], wv_sb[:, c * 7 + dx, :],
                            padt[:, a + d:b + d],
                            start=(nmm == 0), stop=(nmm == 13),
                            skip_group_check=True)
                        nmm += 1
                spsc_yx = wp.tile([56, 56], F32, tag="spscyx")
                nc.scalar.activation(spsc_yx[:], conv_ps[:], AF.Sigmoid,
                                     bias=bcb[:])
                spscT = wp.tile([NT, PT], F32, tag="spscT")
                nc.sync.dma_start(
                    bass.AP(sc_dram, (s % 2) * NP, [[1, NP]]), spsc_yx[:])
                nc.sync.dma_start(
                    spscT[:],
                    bass.AP(sc_dram, (s % 2) * NP, [[112, 28], [1, 112]]))
                tps2 = psA.tile([PT, NT], F32, tag="psa")
                nc.tensor.transpose(tps2[:], spscT[:], ident_f[0:NT, 0:NT])
                spsc = spp.tile([PT, NT], F32, tag="spsc")
                nc.vector.tensor_copy(spsc[:], tps2[:])

                # ---------- out = xg * spatial (in place) + store ----------
                for t in range(NT):
                    v = xbuf[:, s * NT + t, :]
                    nc.scalar.activation(v, v, AF.Copy,
                                         scale=spsc[:, t:t + 1])
                for j in range(NSLAB):
                    base = (s * NP + j * TPS * PT) * C
                    nc.sync.dma_start(
                        bass.AP(out_d, base, [[C, PT], [PT * C, TPS], [1, C]]),
                        xbuf[:, s * NT + j * TPS:s * NT + (j + 1) * TPS, :])

    nc.compile()
    return nc


def _get_compiled():
    global _COMPILED
    if _COMPILED is None:
        _COMPILED = _build()
    return _COMPILED


def _make_wv(wconv):
    # wv[(c*7+dx)*62 + y', x] = wconv[y'-y, dx, c, 0]  (banded, SAME pad in y)
    w = np.asarray(wconv, dtype=np.float32)[:, :, :, 0]    # [dy, dx, c]
    w = w.copy()
    w[:, :, 0] /= C       # fold channel-mean 1/256 into the mean-plane taps
    wv = np.zeros((14, 62, 56), dtype=np.float32)
    idx = np.arange(56)
    for c in range(2):
        for dx in range(7):
            for dy in range(7):
                wv[c * 7 + dx, idx + dy, idx] = w[dy, dx, c]
    return wv.reshape(14 * 62, 56)


def kernel(x, w1, b1, w2, b2, wconv, bconv):
    x = np.asarray(x, dtype=np.float32).reshape(N_CORES, ROWS, C)
    xbf = x.astype(ml_dtypes.bfloat16)
    wv = _make_wv(wconv)

    nc = _get_compiled()
    in_maps = [{
        "x": np.ascontiguousarray(xbf[i]),
        "w1": np.asarray(w1, np.float32),
        "b1": np.asarray(b1, np.float32).reshape(1, R),
        "w2": np.asarray(w2, np.float32),
        "b2": np.asarray(b2, np.float32).reshape(1, C),
        "wv": wv,
        "bconv": np.asarray(bconv, np.float32).reshape(1, 1),
    } for i in range(N_CORES)]
    res = run_bass_kernel_spmd(nc, in_maps, list(range(N_CORES)))
    out = np.stack([np.asarray(res.results[i]["out"]) for i in range(N_CORES)],
                   axis=0)
    return out.astype(np.float32).reshape(B, H, W, C)


# revision 14
# speedup vs baseline: 1.5303x; 1.0243x over previous
import sys

sys.path.insert(0, "/opt/trn_rl_repo")
import numpy as np
import ml_dtypes
import concourse.bass as bass
import concourse.bacc as bacc
import concourse.mybir as mybir
import concourse.tile as tile
from concourse import masks
import concourse.bass_utils as bass_utils

bass_utils.upload_artifacts = lambda tmpdir: "local://" + tmpdir
from concourse.bass_utils import run_bass_kernel_spmd

N_CORES = 8
B, H, W, C, R = 32, 56, 56, 256, 16
BS = B // N_CORES          # 4 samples per core
NP = H * W                 # 3136 pixels per sample
PT = 112                   # partitions per tile (2 image rows)
NT = NP // PT              # 28 tiles per sample
TPS = 7                    # tiles per DMA slab
NSLAB = NT // TPS          # 4 slabs per sample
ROWS = BS * NP             # 12544 rows per core
F32 = mybir.dt.float32
BF16 = mybir.dt.bfloat16
AL = mybir.AluOpType
AF = mybir.ActivationFunctionType
AX = mybir.AxisListType

_COMPILED = None


def _build():
    nc = bacc.Bacc(None, target_bir_lowering=False, num_devices=N_CORES)
    x_d = nc.declare_dram_parameter("x", [ROWS, C], BF16, isOutput=False)
    w1_d = nc.declare_dram_parameter("w1", [C, R], F32, isOutput=False)
    b1_d = nc.declare_dram_parameter("b1", [1, R], F32, isOutput=False)
    w2_d = nc.declare_dram_parameter("w2", [R, C], F32, isOutput=False)
    b2_d = nc.declare_dram_parameter("b2", [1, C], F32, isOutput=False)
    wv_d = nc.declare_dram_parameter("wv", [14 * 62, 56], F32, isOutput=False)
    bc_d = nc.declare_dram_parameter("bconv", [1, 1], F32, isOutput=False)
    out_d = nc.declare_dram_parameter("out", [ROWS, C], BF16, isOutput=True)
    # DRAM bounce buffers for plane-layout rearrangement (ping-pong x2)
    sd_dram = nc.dram_tensor("sd_dram", [2 * 2, NP], F32)
    sc_dram = nc.dram_tensor("sc_dram", [2, NP], F32)

    with tile.TileContext(nc) as tc:
        with tc.tile_pool(name="const", bufs=1) as cp, \
             tc.tile_pool(name="xbuf", bufs=1) as xp, \
             tc.tile_pool(name="maxp", bufs=4) as mp, \
             tc.tile_pool(name="work", bufs=2) as wp, \
             tc.tile_pool(name="sp", bufs=2) as spp, \
             tc.tile_pool(name="psPool", bufs=2, space="PSUM") as psP, \
             tc.tile_pool(name="psA", bufs=2, space="PSUM") as psA, \
             tc.tile_pool(name="psB", bufs=2, space="PSUM") as psB, \
             tc.tile_pool(name="psC", bufs=2, space="PSUM") as psC:

            # ---------- constants ----------
            ident_f = cp.tile([128, 128], F32)
            masks.make_identity(nc, ident_f[:])
            ident_b = cp.tile([128, 128], BF16)
            masks.make_identity(nc, ident_b[:])
            ones_t = cp.tile([PT, 1], BF16)
            nc.gpsimd.memset(ones_t[:], 1.0)
            ones2f = cp.tile([2, PT], F32)
            nc.gpsimd.memset(ones2f[:], 1.0)

            w1t = cp.tile([128, 2 * R], F32)       # [K-chunk, 2*16]
            nc.sync.dma_start(w1t[:, 0:R], w1_d[0:128, :])
            nc.sync.dma_start(w1t[:, R:2 * R], w1_d[128:256, :])
            w2t = cp.tile([R, C], F32)
            nc.sync.dma_start(w2t[:], w2_d[:])
            # 14 band matrices [62,56], one per (channel, dx)
            wv_sb = cp.tile([62, 14, 56], F32)
            nc.sync.dma_start(
                wv_sb[:],
                bass.AP(wv_d, 0, [[56, 62], [62 * 56, 14], [1, 56]]))

            b1r = cp.tile([1, R], F32)
            nc.sync.dma_start(b1r[:], b1_d[:])
            b1b = cp.tile([2, R], F32)
            nc.gpsimd.partition_broadcast(b1b[:], b1r[:], channels=2)
            b2r = cp.tile([1, C], F32)
            nc.sync.dma_start(b2r[:], b2_d[:])
            b2b = cp.tile([2, C], F32)
            nc.gpsimd.partition_broadcast(b2b[:], b2r[:], channels=2)
            bcr = cp.tile([1, 1], F32)
            nc.sync.dma_start(bcr[:], bc_d[:])
            bcb = cp.tile([56, 1], F32)
            nc.gpsimd.partition_broadcast(bcb[:], bcr[:], channels=56)

            # zero-padded conv input planes (borders stay zero), ping-pong x2
            pads = []
            for i in range(2):
                pm = cp.tile([62, 56], F32, name=f"padm{i}")
                px = cp.tile([62, 56], F32, name=f"padx{i}")
                nc.vector.memset(pm[:], 0.0)
                nc.vector.memset(px[:], 0.0)
                pads.append((pm, px))

            # resident x (overwritten in place by xg then by out)
            xbuf = xp.tile([PT, BS * NT, C], BF16)

            # ---------- load all of x ----------
            for s in range(BS):
                for j in range(NSLAB):
                    base = (s * NP + j * TPS * PT) * C
                    nc.sync.dma_start(
                        xbuf[:, s * NT + j * TPS:s * NT + (j + 1) * TPS, :],
                        bass.AP(x_d, base, [[C, PT], [PT * C, TPS], [1, C]]))

            for s in range(BS):
                # ---------- channel pooling ----------
                pool_ps = psP.tile([1, C], F32, tag="pool")
                for t in range(NT):
                    v = xbuf[:, s * NT + t, :]
                    nc.tensor.matmul(
                        pool_ps[:], ones_t[:], v,
                        start=(t == 0), stop=(t == NT - 1),
                        skip_group_check=True)
                scr = wp.tile([PT, 14, C], BF16, tag="scr")
                nc.vector.tensor_tensor(
                    out=scr[:], in0=xbuf[:, s * NT:s * NT + 14, :],
                    in1=xbuf[:, s * NT + 14:s * NT + 28, :], op=AL.max)
                s7 = wp.tile([PT, 7, C], BF16, tag="s7")
                nc.vector.tensor_tensor(
                    out=s7[:], in0=scr[:, 0:7, :], in1=scr[:, 7:14, :],
                    op=AL.max)
                maxacc = wp.tile([PT, C], BF16, tag="maxacc")
                nc.vector.tensor_tensor(out=maxacc[:], in0=s7[:, 0, :],
                                        in1=s7[:, 1, :], op=AL.max)
                for k in range(2, 7):
                    nc.vector.tensor_tensor(out=maxacc[:], in0=maxacc[:],
                                            in1=s7[:, k, :], op=AL.max)

                # ---------- channel MLP ----------
                poolsb = wp.tile([1, C], F32, tag="poolsb")
                nc.scalar.activation(poolsb[:], pool_ps[:],
                                     AF.Copy, scale=1.0 / NP)
                rhs_s = wp.tile([128, 2, 2], F32, tag="rhs")
                for c in range(2):
                    tp = psB.tile([128, 1], F32, tag="psb")
                    nc.tensor.transpose(tp[:], poolsb[:, c * 128:(c + 1) * 128],
                                        ident_f[0:1, 0:1])
                    nc.vector.tensor_copy(rhs_s[:, c, 0:1], tp[:])
                    mt = psA.tile([128, PT], BF16, tag="psa")
                    nc.tensor.transpose(mt[:], maxacc[:, c * 128:(c + 1) * 128],
                                        ident_b[0:PT, 0:PT])
                    nc.vector.reduce_max(rhs_s[:, c, 1:2], mt[:], axis=AX.X)
                h_ps = psB.tile([2, R], F32, tag="psb")
                nc.tensor.matmul(h_ps[:], rhs_s[:, 0, :], w1t[:, 0:R],
                                 start=True, stop=False)
                nc.tensor.matmul(h_ps[:], rhs_s[:, 1, :], w1t[:, R:2 * R],
                                 start=False, stop=True)
                hb = wp.tile([2, R], F32, tag="hb")
                nc.vector.tensor_tensor(out=hb[:], in0=h_ps[:], in1=b1b[:],
                                        op=AL.add)
                hr = wp.tile([2, R], F32, tag="hr")
                nc.scalar.activation(hr[:], hb[:], AF.Relu)
                hT_ps = psB.tile([R, 2], F32, tag="psb")
                nc.tensor.transpose(hT_ps[:], hr[:], ident_f[0:2, 0:2])
                hT = wp.tile([R, 2], F32, tag="hT")
                nc.vector.tensor_copy(hT[:], hT_ps[:])
                co_ps = psB.tile([2, C], F32, tag="psb")
                nc.tensor.matmul(co_ps[:], hT[:], w2t[:], start=True, stop=True)
                co_sb = wp.tile([2, C], F32, tag="co")
                nc.vector.tensor_tensor(out=co_sb[:], in0=co_ps[:], in1=b2b[:],
                                        op=AL.add)
                sig = wp.tile([2, C], F32, tag="sig")
                nc.scalar.activation(sig[:], co_sb[:], AF.Sigmoid)
                cb_ps = psB.tile([PT, C], F32, tag="psb")
                nc.tensor.matmul(cb_ps[:], ones2f[:], sig[:],
                                 start=True, stop=True)
                cbb = wp.tile([PT, C], BF16, tag="cbb")
                nc.vector.tensor_copy(cbb[:], cb_ps[:])

                # ---------- xg (in place) + spatial stats ----------
                spx = spp.tile([PT, NT], F32, tag="spx")
                spm = spp.tile([PT, NT], F32, tag="spm")
                for t in range(NT):
                    v = xbuf[:, s * NT + t, :]
                    nc.vector.tensor_tensor(out=v, in0=v, in1=cbb[:],
                                            op=AL.mult)
                for j in range(NSLAB):
                    slab = xbuf[:, s * NT + j * TPS:s * NT + (j + 1) * TPS, :]
                    nc.vector.reduce_max(spx[:, j * TPS:(j + 1) * TPS],
                                         slab, axis=AX.X)
                    nc.vector.reduce_sum(spm[:, j * TPS:(j + 1) * TPS],
                                         slab, axis=AX.X)

                # ---------- 7x7x2 conv via banded matmuls ----------
                padm, padx = pads[s % 2]
                for ci, (plane, padt) in enumerate(((spm, padm), (spx, padx))):
                    tps = psA.tile([NT, PT], F32, tag="psa")
                    nc.tensor.transpose(tps[:], plane[:], ident_f[0:PT, 0:PT])
                    smT = wp.tile([NT, PT], F32, tag="smT")
                    nc.vector.tensor_copy(smT[:], tps[:])
                    row = (s % 2) * 2 + ci
                    nc.sync.dma_start(
                        bass.AP(sd_dram, row * NP, [[112, 28], [1, 112]]),
                        smT[:])
                    nc.sync.dma_start(
                        padt[3:59, :],
                        bass.AP(sd_dram, row * NP, [[56, 56], [1, 56]]))
                conv_ps = psC.tile([56, 56], F32, tag="conv")
                dx_order = [3, 0, 1, 2, 4, 5, 6]
                nmm = 0
                for c, padt in ((0, padm), (1, padx)):
                    for dx in (dx_order if c == 0 else range(7)):
                        d = dx - 3
                        a = max(0, -d)
                        b = 56 - max(0, d)
                        nc.tensor.matmul(
                            conv_ps[0:56, a:b], wv_sb[:, c * 7 + dx, :],
                            padt[:, a + d:b + d],
                            start=(nmm == 0), stop=(nmm == 13),
                            skip_group_check=True)
                        nmm += 1
                spsc_yx = wp.tile([56, 56], F32, tag="spscyx")
                nc.scalar.activation(spsc_yx[:], conv_ps[:], AF.Sigmoid,
                                     bias=bcb[:])
                spscT = wp.tile([NT, PT], F32, tag="spscT")
                nc.sync.dma_start(
                    bass.AP(sc_dram, (s % 2) * NP, [[1, NP]]), spsc_yx[:])
                nc.sync.dma_start(
                    spscT[:],
                    bass.AP(sc_dram, (s % 2) * NP, [[112, 28], [1, 112]]))
                tps2 = psA.tile([PT, NT], F32, tag="psa")
                nc.tensor.transpose(tps2[:], spscT[:], ident_f[0:NT, 0:NT])
                spsc = spp.tile([PT, NT], F32, tag="spsc")
                nc.vector.tensor_copy(spsc[:], tps2[:])

                # ---------- out = xg * spatial (in place) + store ----------
                for t in range(NT):
                    v = xbuf[:, s * NT + t, :]
                    nc.scalar.activation(v, v, AF.Copy,
                                         scale=spsc[:, t:t + 1])
                for j in range(NSLAB):
                    base = (s * NP + j * TPS * PT) * C
                    nc.sync.dma_start(
                        bass.AP(out_d, base, [[C, PT], [PT * C, TPS], [1, C]]),
                        xbuf[:, s * NT + j * TPS:s * NT + (j + 1) * TPS, :])

    nc.compile()
    return nc


def _get_compiled():
    global _COMPILED
    if _COMPILED is None:
        _COMPILED = _build()
    return _COMPILED


def _make_wv(wconv):
    # wv[(c*7+dx)*62 + y', x] = wconv[y'-y, dx, c, 0]  (banded, SAME pad in y)
    w = np.asarray(wconv, dtype=np.float32)[:, :, :, 0]    # [dy, dx, c]
    w = w.copy()
    w[:, :, 0] /= C       # fold channel-mean 1/256 into the mean-plane taps
    wv = np.zeros((14, 62, 56), dtype=np.float32)
    idx = np.arange(56)
    for c in range(2):
        for dx in range(7):
            for dy in range(7):
                wv[c * 7 + dx, idx + dy, idx] = w[dy, dx, c]
    return wv.reshape(14 * 62, 56)


def kernel(x, w1, b1, w2, b2, wconv, bconv):
    x = np.asarray(x, dtype=np.float32).reshape(N_CORES, ROWS, C)
    xbf = x.astype(ml_dtypes.bfloat16)
    wv = _make_wv(wconv)

    nc = _get_compiled()
    in_maps = [{
        "x": np.ascontiguousarray(xbf[i]),
        "w1": np.asarray(w1, np.float32),
        "b1": np.asarray(b1, np.float32).reshape(1, R),
        "w2": np.asarray(w2, np.float32),
        "b2": np.asarray(b2, np.float32).reshape(1, C),
        "wv": wv,
        "bconv": np.asarray(bconv, np.float32).reshape(1, 1),
    } for i in range(N_CORES)]
    res = run_bass_kernel_spmd(nc, in_maps, list(range(N_CORES)))
    out = np.stack([np.asarray(res.results[i]["out"]) for i in range(N_CORES)],
                   axis=0)
    return out.astype(np.float32).reshape(B, H, W, C)
